# revision 7
# baseline (speedup 1.0000x reference)
"""Trainium2 Bass kernel for a 2-layer GCN + linear classifier (PyG GCNConv style).

Self-contained: hardcodes the 8-core sharding strategy; all graph/index
preprocessing is host-side numpy, all FLOPs on x run on device.

Sharding: nodes are split into 8 contiguous blocks (one per core, padded to
a multiple of 128; slot 0 / tail slots are guaranteed-zero pad rows). Per
GCN layer each core computes its block's dense transform (bf16 PE matmuls,
f32 PSUM), pre-scales rows by dinv, and two AllGathers materialize the full
bf16 node table in every core's HBM, split into halves A/B so gather
indices fit int16. Self-loop edges are folded into the gather grids. Each
core aggregates its destinations' in-edges with gpsimd dma_gather (256B
bf16 rows, <=1024 idxs/instruction — hard ucode limit — over 4 SWDGE
queues) + DVE tree reductions, in two passes by table half. Destination
lanes are degree-sorted per (core, half); pass-A partials are re-permuted
into pass-B lane order via one small gather that overlaps pass B, so the
merge (+bias, x dinv, ReLU) runs per-tile as pass B completes and the next
layer's dense transform / AllGather pipeline into the gather tail. Layer 2
operates in pass-B-of-layer-1 row order; the final row permutation is
undone on the host. The bottleneck is Pool-engine descriptor generation at
~2.1 ns/index.
"""

import sys
import types

import numpy as np


def _setup_env():
    if "/opt/trn_rl_repo" not in sys.path:
        sys.path.insert(0, "/opt/trn_rl_repo")
    if "antenv.axon_hooks" not in sys.modules:
        try:
            from trn_agent_boot.trn_boot import _ntff_profile_via_ctypes

            _hook = _ntff_profile_via_ctypes("/opt/axon/libaxon_pjrt.so")
        except Exception:
            _hook = None
        _mod = types.ModuleType("antenv.axon_hooks")
        _mod.get_axon_ntff_profile_hook = lambda: _hook
        _mod.set_axon_ntff_profile_hook = lambda h: None
        sys.modules["antenv.axon_hooks"] = _mod


_setup_env()

import ml_dtypes  # noqa: E402
from concourse import bacc, bass, mybir, tile  # noqa: E402
import concourse.bass_utils as bass_utils  # noqa: E402
from concourse.bass_utils import run_bass_kernel_spmd  # noqa: E402
from concourse.masks import make_identity  # noqa: E402

bass_utils.upload_artifacts = lambda tmpdir: tmpdir

# --- queue-aware DMASW semaphore lane assignment -----------------------------
# Tile assigns Pool-engine DMA instructions to the 8 DMASW semaphore lanes
# round-robin in *scheduled* order, but each lane gets locked to the SWDGE
# queue of the first instruction using it. With multi-queue dma_gather this
# races; pin each queue to its own lane subset instead.
import concourse.tile_sem_assignment as _tsa  # noqa: E402
from concourse.bass_isa import UserSyncedRemoteDMADescs as _URD  # noqa: E402
from concourse.tile_sem_assignment import DMAInst as _DMAInst  # noqa: E402

_orig_assign_tick = _tsa.TileClockTick._assign_tick


def _queue_aware_assign_tick(self, inst):
    if (
        isinstance(inst, _DMAInst)
        and not isinstance(inst, _URD)
        and inst.engine == mybir.EngineType.Pool
    ):
        q = getattr(inst, "queue_num", 0) or 0
        lanes = max(1, self.swdge_sem_count // NQ)
        rot = self.__dict__.setdefault("_q_lane_rot", {})
        r = rot.get(q, 0)
        self.next_sw_dma_idx = (q * lanes + r) % self.swdge_sem_count
        rot[q] = (r + 1) % lanes
    return _orig_assign_tick(self, inst)


_tsa.TileClockTick._assign_tick = _queue_aware_assign_tick
# -----------------------------------------------------------------------------

import os  # noqa: E402

N_CORES = 8
P = 128
CHUNK = int(os.environ.get("KCHUNK", "8"))   # slabs per dma_gather instruction
NQ = int(os.environ.get("KNQ", "2"))         # SWDGE queues
GBUFS = int(os.environ.get("KGBUFS", "8"))
SCAP = int(os.environ.get("KSCAP", "40"))    # staging slabs per group
QCH = int(os.environ.get("KQCH", "12"))      # out-DMA tile chunk
KPREP = int(os.environ.get("KPREP", "0"))    # prepare_only prefix instrs (L1 passA)

dt = mybir.dt
BF16 = ml_dtypes.bfloat16


# ----------------------------------------------------------------------------
# Host-side preprocessing
# ----------------------------------------------------------------------------

def _wrap16(flat: np.ndarray) -> np.ndarray:
    """Lay out an index list in dma_gather's [128, n/16] wrapped format."""
    n = flat.shape[0]
    assert n % 16 == 0
    w = flat.reshape(n // 16, 16).T.astype(np.int16)  # [16, n//16]
    return np.tile(w, (8, 1))  # replicate across the 8 groups of 16 partitions


def _build_layer_geom(base_pos, s_core, s_slot, d_core, d_slot, dinv_base,
                      dims):
    """Geometry for one GCN layer.

    base_pos[r, j]: canonical slot j of core r -> row position in this
    layer's base space (= the order in which h/table rows are laid out).
    dinv_base[r, p]: dinv of core r's node at base position p.

    Returns per-core grids/maps and the next layer's base_pos (= pass-B
    lane order, in which this layer's outputs are produced).
    """
    BLK, MT, HA, HB, BLK_RAW = (dims["BLK"], dims["MT"], dims["HA"],
                                dims["HB"], dims["BLK_RAW"])
    MTA = dims["MTA"]

    sp = base_pos[s_core, s_slot]                 # source base positions
    s_half = (sp >= HA).astype(np.int8)
    s_row = np.where(s_half == 1,
                     s_core * HB + (sp - HA),
                     s_core * HA + sp)            # row within half table
    dp = base_pos[d_core, d_slot]                 # dst base positions

    # per-core per-half degree sorts
    percore = []
    KA = np.zeros(MT, dtype=np.int64)
    KB = np.zeros(MT, dtype=np.int64)
    for r in range(N_CORES):
        m = d_core == r
        halves = []
        p0 = base_pos[r, 0]                       # guaranteed-zero pad slot
        pt = base_pos[r, BLK_RAW + 1] if BLK_RAW + 1 < BLK else None
        for h in (0, 1):
            mh = m & (s_half == h)
            cnt = np.bincount(dp[mh], minlength=BLK)
            key = cnt.astype(np.float64)
            if h == 1:
                # force zero rows of the *next* layer's table to fixed spots:
                # pad slot 0 -> pos 0 (half A), one tail pad -> pos BLK-1 (B)
                key[p0] = -1.0
                if pt is not None:
                    key[pt] = np.inf
            perm = np.argsort(key, kind="stable")  # perm[pos] = base position
            inv = np.empty(BLK, dtype=np.int64)
            inv[perm] = np.arange(BLK)
            scnt = cnt[perm]
            Kt = scnt.reshape(MT, P).max(axis=1)
            if h == 0:
                KA = np.maximum(KA, Kt)
            else:
                KB = np.maximum(KB, Kt)
            halves.append(dict(perm=perm, inv=inv))
        percore.append(halves)

    WA, WB = int(KA.sum()), int(KB.sum())
    offA = np.concatenate([[0], np.cumsum(KA)[:-1]])
    offB = np.concatenate([[0], np.cumsum(KB)[:-1]])

    # grids (linear slab streams), fillers point at guaranteed-zero rows
    grids = []
    for r in range(N_CORES):
        m = d_core == r
        cg = []
        for h, (K, off, W) in enumerate(((KA, offA, WA), (KB, offB, WB))):
            if W == 0:
                cg.append(None)
                continue
            zrow = r * HA if h == 0 else r * HB + HB - 1
            grid = np.full((W, P), zrow, dtype=np.int64)
            mh = m & (s_half == h)
            pos = percore[r][h]["inv"][dp[mh]]
            rows = s_row[mh]
            order = np.argsort(pos, kind="stable")
            pos_s = pos[order]
            rows_s = rows[order]
            counts = np.bincount(pos_s, minlength=BLK)
            starts = np.concatenate([[0], np.cumsum(counts)[:-1]])
            k = np.arange(len(pos_s)) - starts[pos_s]
            tile_i = pos_s // P
            lane = pos_s % P
            grid[off[tile_i] + k, lane] = rows_s
            cg.append(grid)
        grids.append(cg)

    # group segmentation for the staged tree-reduce (shared geometry)
    def segments(K, off, W):
        segs = []  # per group: list of (tile, s0_in_group, length, first)
        for g0 in range(0, W, SCAP):
            g1 = min(W, g0 + SCAP)
            out = []
            t = int(np.searchsorted(off, g0, side="right")) - 1
            while t < MT and off[t] + K[t] <= g0:
                t += 1
            s = g0
            while s < g1 and t < MT:
                e = min(g1, int(off[t] + K[t]))
                if e > s:
                    out.append((t, s - g0, e - s, s == off[t]))
                s = e
                t += 1
            segs.append(out)
        return segs

    segA = segments(KA, offA, WA)
    segB = segments(KB, offB, WB)

    # pass-A -> pass-B lane-space map (per core), wrapped for dma_gather
    maps = []
    for r in range(N_CORES):
        permB = percore[r][1]["perm"]
        invA = percore[r][0]["inv"]
        maps.append(invA[permB])  # mapAB[pB] = pass-A position of that node

    # next layer's base space = pass-B order of this layer
    base_pos2 = np.empty_like(base_pos)
    dinv_base2 = np.empty_like(dinv_base)
    for r in range(N_CORES):
        invB = percore[r][1]["inv"]
        permB = percore[r][1]["perm"]
        base_pos2[r] = invB[base_pos[r]]
        dinv_base2[r] = dinv_base[r][permB]

    return dict(KA=tuple(int(x) for x in KA), KB=tuple(int(x) for x in KB),
                WA=WA, WB=WB, segA=segA, segB=segB, grids=grids, maps=maps,
                base_pos2=base_pos2, dinv_base2=dinv_base2)


def _preprocess(x, edge_index, W1, b1, W2, b2, Wfc, bfc):
    N, IN = x.shape
    HID = W1.shape[1]
    CLS = Wfc.shape[1]
    assert IN % P == 0 and HID == P

    BLK_RAW = -(-N // N_CORES)
    BLK = -(-BLK_RAW // P) * P
    assert BLK_RAW + 2 <= BLK, "need >=2 pad slots per block"
    MT = BLK // P
    # smallest half-A that keeps half-B table rows within int16: the first
    # AllGather covers less data, so pass-A gathers start earlier.
    max_half = 32767 // (N_CORES * P)
    MTA = MT - max_half if MT - max_half > 0 else MT // 2
    kmta = int(os.environ.get("KMTA", "0"))
    if kmta:
        MTA = kmta
    HA = MTA * P
    HB = BLK - HA
    NROWSA = N_CORES * HA
    NROWSB = N_CORES * HB
    assert NROWSA < 32768 and NROWSB < 32768
    assert BLK_RAW + 1 >= HA, "tail pads must land in half B"
    dims = dict(BLK=BLK, MT=MT, MTA=MTA, HA=HA, HB=HB, BLK_RAW=BLK_RAW)

    # edge list WITH self loops
    loop = np.arange(N, dtype=np.int64)
    src = np.concatenate([edge_index[0].astype(np.int64), loop])
    dst = np.concatenate([edge_index[1].astype(np.int64), loop])

    deg = np.bincount(dst, minlength=N).astype(np.float64)
    dinv = np.where(deg > 0, 1.0 / np.sqrt(np.maximum(deg, 1.0)), 0.0)
    dinv = dinv.astype(np.float32)

    s_core = src // BLK_RAW
    s_slot = 1 + src % BLK_RAW
    d_core = dst // BLK_RAW
    d_slot = 1 + dst % BLK_RAW

    # canonical (layer-1) base space: position = slot
    base_pos1 = np.tile(np.arange(BLK, dtype=np.int64)[None, :], (N_CORES, 1))
    dinv_base1 = np.zeros((N_CORES, BLK), dtype=np.float32)
    for r in range(N_CORES):
        lo = r * BLK_RAW
        hi = min(N, (r + 1) * BLK_RAW)
        if hi > lo:
            dinv_base1[r, 1:1 + hi - lo] = dinv[lo:hi]

    g1 = _build_layer_geom(base_pos1, s_core, s_slot, d_core, d_slot,
                           dinv_base1, dims)
    g2 = _build_layer_geom(g1["base_pos2"], s_core, s_slot, d_core, d_slot,
                           g1["dinv_base2"], dims)

    # output row of canonical slot j = pass-B-of-layer-2 position
    out_pos = g2["base_pos2"]  # [r, slot] -> final row

    # per-core input tensors
    b1r = np.tile(np.asarray(b1, np.float32)[None, :], (P, 1)).astype(BF16)
    b2r = np.tile(np.asarray(b2, np.float32)[None, :], (P, 1)).astype(BF16)
    bfcr = np.tile(np.asarray(bfc, np.float32)[None, :], (P, 1))
    w1 = np.asarray(W1, np.float32).astype(BF16)
    w2 = np.asarray(W2, np.float32).astype(BF16)
    wfc = np.asarray(Wfc, np.float32).astype(BF16)

    in_maps = []
    for r in range(N_CORES):
        lo = r * BLK_RAW
        hi = min(N, (r + 1) * BLK_RAW)
        xb = np.zeros((BLK, IN), dtype=np.float32)
        if hi > lo:
            xb[1:1 + hi - lo] = x[lo:hi]
        xt = np.ascontiguousarray(xb.T).astype(BF16)

        def dvt(v):  # [BLK] -> [P, MT]
            return np.ascontiguousarray(v.reshape(MT, P).T.copy())

        im = {
            "xt": xt, "w1": w1, "w2": w2, "wfc": wfc,
            "b1r": b1r, "b2r": b2r, "bfcr": bfcr,
            "dv1": dvt(dinv_base1[r]),
            "dvb1": dvt(g1["dinv_base2"][r]),
            "dvb2": dvt(g2["dinv_base2"][r]),
        }
        for li, g in ((1, g1), (2, g2)):
            for h, nm in ((0, "a"), (1, "b")):
                grid = g["grids"][r][h]
                if grid is not None:
                    im[f"i{li}{nm}"] = np.ascontiguousarray(
                        _wrap16(grid.reshape(-1)))
            im[f"m{li}"] = np.ascontiguousarray(_wrap16(g["maps"][r]))
        in_maps.append(im)

    meta = dict(N=N, IN=IN, HID=HID, CLS=CLS, BLK=BLK, BLK_RAW=BLK_RAW,
                MT=MT, MTA=MTA, NROWSA=NROWSA, NROWSB=NROWSB,
                KA1=g1["KA"], KB1=g1["KB"], KA2=g2["KA"], KB2=g2["KB"],
                segA1=g1["segA"], segB1=g1["segB"],
                segA2=g2["segA"], segB2=g2["segB"],
                WA1=g1["WA"], WB1=g1["WB"], WA2=g2["WA"], WB2=g2["WB"],
                out_pos=out_pos)
    return in_maps, meta


# ----------------------------------------------------------------------------
# Device graph
# ----------------------------------------------------------------------------

def _tree_seg(nc, g, s0, n, out_ap, acc, tpool):
    """Sum g[:, s0:s0+n, :] into out_ap (add into it if acc)."""
    HID = out_ap.shape[-1]
    if n == 1:
        if acc:
            nc.vector.tensor_add(out_ap, out_ap, g[:, s0, :])
        else:
            nc.vector.tensor_copy(out_ap, g[:, s0, :])
        return
    while n > 2:
        if n % 2 == 1:
            nc.vector.tensor_add(g[:, s0, :], g[:, s0, :], g[:, s0 + n - 1, :])
            n -= 1
            if n == 2:
                break
        h = n // 2
        nc.vector.tensor_add(g[:, s0:s0 + h, :], g[:, s0:s0 + h, :],
                             g[:, s0 + h:s0 + 2 * h, :])
        n = h
    if acc:
        nc.vector.tensor_add(g[:, s0, :], g[:, s0, :], g[:, s0 + 1, :])
        nc.vector.tensor_add(out_ap, out_ap, g[:, s0, :])
    else:
        nc.vector.tensor_add(out_ap, g[:, s0, :], g[:, s0 + 1, :])


def _build(meta):
    IN, HID, CLS = meta["IN"], meta["HID"], meta["CLS"]
    BLK, MT, MTA = meta["BLK"], meta["MT"], meta["MTA"]
    NROWSA, NROWSB = meta["NROWSA"], meta["NROWSB"]
    KC = IN // P

    nc = bacc.Bacc("TRN2", target_bir_lowering=False, debug=False,
                   num_devices=N_CORES, num_swdge_queues=NQ)

    xt = nc.dram_tensor("xt", [IN, BLK], dt.bfloat16, kind="ExternalInput")
    w1 = nc.dram_tensor("w1", [IN, HID], dt.bfloat16, kind="ExternalInput")
    w2 = nc.dram_tensor("w2", [HID, HID], dt.bfloat16, kind="ExternalInput")
    wfc = nc.dram_tensor("wfc", [HID, CLS], dt.bfloat16, kind="ExternalInput")
    b1r = nc.dram_tensor("b1r", [P, HID], dt.bfloat16, kind="ExternalInput")
    b2r = nc.dram_tensor("b2r", [P, HID], dt.bfloat16, kind="ExternalInput")
    bfcr = nc.dram_tensor("bfcr", [P, CLS], dt.float32, kind="ExternalInput")
    dv1 = nc.dram_tensor("dv1", [P, MT], dt.float32, kind="ExternalInput")
    dvb1 = nc.dram_tensor("dvb1", [P, MT], dt.float32, kind="ExternalInput")
    dvb2 = nc.dram_tensor("dvb2", [P, MT], dt.float32, kind="ExternalInput")
    idxt = {}
    for li in (1, 2):
        for nm, W in (("a", meta[f"WA{li}"]), ("b", meta[f"WB{li}"])):
            if W:
                idxt[f"i{li}{nm}"] = nc.dram_tensor(
                    f"i{li}{nm}", [P, W * 8], dt.int16, kind="ExternalInput")
        idxt[f"m{li}"] = nc.dram_tensor(
            f"m{li}", [P, BLK // 16], dt.int16, kind="ExternalInput")
    out = nc.dram_tensor("out", [BLK, CLS], dt.float32, kind="ExternalOutput")

    with tile.TileContext(nc) as tc:
        with (
            tc.tile_pool(name="const", bufs=1) as cpool,
            tc.tile_pool(name="idx", bufs=1) as ipool,
            tc.tile_pool(name="hs", bufs=1) as hspool,
            tc.tile_pool(name="pa", bufs=1) as papool,
            tc.tile_pool(name="acc", bufs=1) as accpool,
            tc.tile_pool(name="pb", bufs=2) as pbpool,
            tc.tile_pool(name="outs", bufs=1) as opool,
            tc.tile_pool(name="xload", bufs=3) as xpool,
            tc.tile_pool(name="gbuf", bufs=GBUFS) as gpool,
            tc.tile_pool(name="lhsT", bufs=3) as tpool,
            tc.tile_pool(name="ps", bufs=3, space="PSUM") as pspool,
            tc.tile_pool(name="pst", bufs=2, space="PSUM") as pstpool,
            tc.tile_pool(name="dram", bufs=1, space="DRAM") as dpool,
        ):
            # ---- constants ----
            w1sb = cpool.tile([P, KC, HID], dt.bfloat16, tag="w1")
            nc.sync.dma_start(out=w1sb[:], in_=w1[:].rearrange("(c k) h -> k c h", k=P))
            w2sb = cpool.tile([P, HID], dt.bfloat16, tag="w2")
            nc.sync.dma_start(out=w2sb[:], in_=w2[:])
            wfcsb = cpool.tile([P, CLS], dt.bfloat16, tag="wfc")
            nc.sync.dma_start(out=wfcsb[:], in_=wfc[:])
            b1sb = cpool.tile([P, HID], dt.bfloat16, tag="b1")
            nc.sync.dma_start(out=b1sb[:], in_=b1r[:])
            b2sb = cpool.tile([P, HID], dt.bfloat16, tag="b2")
            nc.sync.dma_start(out=b2sb[:], in_=b2r[:])
            bfcsb = cpool.tile([P, CLS], dt.float32, tag="bfc")
            nc.sync.dma_start(out=bfcsb[:], in_=bfcr[:])
            dvsb = {}
            for nm, t in (("dv1", dv1), ("dvb1", dvb1), ("dvb2", dvb2)):
                s = cpool.tile([P, MT], dt.float32, tag=nm)
                nc.sync.dma_start(out=s[:], in_=t[:])
                dvsb[nm] = s
            ident = cpool.tile([P, P], dt.float32, tag="ident")
            make_identity(nc, ident[:])
            identb = cpool.tile([P, P], dt.bfloat16, tag="identb")
            nc.vector.tensor_copy(identb[:], ident[:])

            # Index buffers are shared between layers (layer-2 tables stream
            # in mid-run once layer-1 passes finish with the buffer).
            WGA = 8 * max(meta["WA1"], meta["WA2"])
            WGB = 8 * max(meta["WB1"], meta["WB2"])
            isb = {}

            def load_idx(nm, tag, width):
                t = idxt.get(nm)
                if t is None:
                    return
                s = ipool.tile([P, width], dt.int16, tag=tag)
                # Activation-engine HWDGE queue: keeps ~7MB of index tables
                # from delaying the layer-1 x tile loads on the Sync queue.
                nc.scalar.dma_start(out=s[:, :t.shape[1]], in_=t[:])
                isb[nm] = s

            load_idx("i1a", "ga", WGA)
            load_idx("i1b", "gb", WGB)
            load_idx("m1", "m1", BLK // 16)
            load_idx("m2", "m2", BLK // 16)

            self_q = [0]  # rotating SWDGE queue assignment
            outsb = opool.tile([P, MT, CLS], dt.float32, tag="outsb")
            out_flushed = [0]

            def flush_out(upto):
                if upto > out_flushed[0]:
                    m0 = out_flushed[0]
                    nc.sync.dma_start(
                        out=out[:].rearrange("(t p) c -> p t c", p=P)[:, m0:upto, :],
                        in_=outsb[:, m0:upto, :],
                    )
                    out_flushed[0] = upto

            prep_sems = ([nc.alloc_semaphore(f"gprep{q}") for q in range(NQ)]
                         if KPREP > 0 else None)

            def run_pass(Wtot, segs, isbuf, tview, part, Ks, prep_n=0):
                # memset zero-K tiles once
                for t in range(MT):
                    if Ks[t] == 0:
                        nc.vector.memset(part[:, t, :], 0.0)
                emitted = [0]
                pending = set()
                for gi, seglist in enumerate(segs):
                    g0 = gi * SCAP
                    glen = min(SCAP, Wtot - g0)
                    gt = gpool.tile([P, SCAP, HID], dt.bfloat16, tag="g")
                    g = gt[:]
                    s0 = 0
                    while s0 < glen:
                        kc = min(CHUNK, glen - s0)
                        o0 = g0 + s0
                        q = self_q[0] % NQ
                        if emitted[0] < prep_n:
                            nc.gpsimd.dma_gather(
                                out_ap=g[:, s0:s0 + kc, :],
                                in_ap=tview,
                                idxs_ap=isbuf[:, o0 * 8:(o0 + kc) * 8],
                                num_idxs=kc * P,
                                num_idxs_reg=kc * P,
                                elem_size=HID,
                                queue_num=q,
                                prepare_only=True,
                                sem=prep_sems[q],
                            )
                            pending.add(q)
                        else:
                            if pending:
                                for qq in sorted(pending):
                                    nc.gpsimd.trigger_dma(count=None,
                                                          queue_num=qq)
                                pending.clear()
                            nc.gpsimd.dma_gather(
                                out_ap=g[:, s0:s0 + kc, :],
                                in_ap=tview,
                                idxs_ap=isbuf[:, o0 * 8:(o0 + kc) * 8],
                                num_idxs=kc * P,
                                num_idxs_reg=kc * P,
                                elem_size=HID,
                                queue_num=q,
                            )
                        emitted[0] += 1
                        self_q[0] += 1
                        s0 += kc
                    for (t, ss, ln, first) in seglist:
                        _tree_seg(nc, g, ss, ln, part[:, t, :],
                                  not first, tpool)
                    yield gi
                if pending:
                    for qq in sorted(pending):
                        nc.gpsimd.trigger_dma(count=None, queue_num=qq)
                    pending.clear()

            def emit_dense2_tile(m, h_prev, hs2):
                pst = pstpool.tile([P, P], dt.bfloat16, tag="tr")
                nc.tensor.transpose(pst[:], h_prev[:, m, :], identb[:])
                hT = tpool.tile([P, P], dt.bfloat16, tag="hT")
                nc.scalar.copy(hT[:], pst[:])
                ps = pspool.tile([P, HID], dt.float32, tag="mm")
                nc.tensor.matmul(ps[:], hT[:], w2sb[:], start=True, stop=True)
                nc.vector.tensor_scalar_mul(hs2[:, m, :], ps[:],
                                            dvsb["dvb1"][:, m:m + 1])

            def emit_table_half(layer, hseg, hs):
                t0, t1 = (0, MTA) if hseg == 0 else (MTA, MT)
                nrows = NROWSA if hseg == 0 else NROWSB
                if t1 == t0:
                    return None
                agin = dpool.tile([(t1 - t0) * P, HID], dt.bfloat16,
                                  tag=f"agin{layer}{hseg}")
                # Activation-engine HWDGE queue: avoids queueing behind the
                # x-tile loads on Sync.
                nc.scalar.dma_start(
                    out=agin[:].rearrange("(t p) h -> p t h", p=P),
                    in_=hs[:, t0:t1, :],
                )
                tbl = dpool.tile([nrows, HID], dt.bfloat16,
                                 tag=f"table{layer}{hseg}",
                                 addr_space="Shared")
                nc.gpsimd.collective_compute(
                    "AllGather",
                    mybir.AluOpType.bypass,
                    replica_groups=[list(range(N_CORES))],
                    ins=[agin[:].opt()],
                    outs=[tbl[:].opt()],
                )
                return tbl

            def emit_canon(layer, partA):
                pdA = dpool.tile([BLK, HID], dt.bfloat16, tag=f"pd{layer}")
                nc.sync.dma_start(
                    out=pdA[:].rearrange("(t p) h -> p t h", p=P), in_=partA[:]
                )
                accA = accpool.tile([P, MT, HID], dt.bfloat16, tag="acc")
                msb = isb[f"m{layer}"]
                for c0 in range(0, MT, CHUNK):
                    cc = min(CHUNK, MT - c0)
                    nc.gpsimd.dma_gather(
                        out_ap=accA[:, c0:c0 + cc, :], in_ap=pdA[:],
                        idxs_ap=msb[:, c0 * 8:(c0 + cc) * 8],
                        num_idxs=cc * P, num_idxs_reg=cc * P,
                        elem_size=HID, queue_num=self_q[0] % NQ,
                    )
                    self_q[0] += 1
                return accA

            MCH = int(os.environ.get("KMCH", "10"))  # merge chunk width (tiles)

            def run_passB(layer, partB, accA, tview, on_chunk, on_progress):
                dmrg = dvsb["dvb1" if layer == 1 else "dvb2"]
                bsb = b1sb if layer == 1 else b2sb
                KB = meta[f"KB{layer}"]
                segB = meta[f"segB{layer}"]
                WB = meta[f"WB{layer}"]
                merged_upto = [0]

                def merge_chunk(c0, c1):
                    w = c1 - c0
                    sl = slice(c0, c1)
                    nc.vector.tensor_add(partB[:, sl, :], partB[:, sl, :],
                                         accA[:, sl, :])
                    dv3 = dmrg[:, sl].to_broadcast([P, w, HID])
                    nc.vector.tensor_tensor(partB[:, sl, :], partB[:, sl, :],
                                            dv3, op=mybir.AluOpType.mult)
                    b3 = bsb[:].rearrange("p (o h) -> p o h", o=1).to_broadcast(
                        [P, w, HID])
                    nc.vector.tensor_tensor(partB[:, sl, :], partB[:, sl, :],
                                            b3, op=mybir.AluOpType.add)
                    nc.scalar.activation(partB[:, sl, :], partB[:, sl, :],
                                         mybir.ActivationFunctionType.Relu)
                    on_chunk(c0, c1)

                def merge_ready(done_tiles):
                    while merged_upto[0] < done_tiles:
                        c0 = merged_upto[0]
                        c1 = min(c0 + MCH, MT)
                        if c1 > done_tiles:
                            break
                        merge_chunk(c0, c1)
                        merged_upto[0] = c1
                        on_progress(merged_upto[0])

                offB = np.concatenate([[0], np.cumsum(KB)[:-1]])
                if WB:
                    for gi in run_pass(WB, segB, isb[f"i{layer}b"],
                                       tview, partB, KB):
                        done_upto = (gi + 1) * SCAP
                        nd = 0
                        while nd < MT and offB[nd] + KB[nd] <= done_upto:
                            nd += 1
                        merge_ready(nd)
                else:
                    for t in range(MT):
                        nc.vector.memset(partB[:, t, :], 0.0)
                merge_ready(MT)

            def run_passA(layer, tview):
                KA = meta[f"KA{layer}"]
                segA = meta[f"segA{layer}"]
                WA = meta[f"WA{layer}"]
                partA = papool.tile([P, MT, HID], dt.bfloat16, tag="pa")
                if WA:
                    pn = KPREP if layer == 1 else 0
                    for _ in run_pass(WA, segA, isb[f"i{layer}a"],
                                      tview, partA, KA, prep_n=pn):
                        pass
                else:
                    for t in range(MT):
                        nc.vector.memset(partA[:, t, :], 0.0)
                return partA

            # ================= layer 1 =================
            hs1 = hspool.tile([P, MT, HID], dt.bfloat16, tag="hs")
            for m in range(MT):
                ps = pspool.tile([P, HID], dt.float32, tag="mm")
                xm = xpool.tile([P, KC, P], dt.bfloat16, tag="x")
                nc.sync.dma_start(
                    out=xm[:],
                    in_=xt[:].rearrange("(c k) m -> k c m", k=P)[
                        :, :, m * P:(m + 1) * P
                    ],
                )
                for c in range(KC):
                    nc.tensor.matmul(ps[:], xm[:, c, :], w1sb[:, c, :],
                                     start=(c == 0), stop=(c == KC - 1))
                nc.vector.tensor_scalar_mul(hs1[:, m, :], ps[:],
                                            dvsb["dv1"][:, m:m + 1])
            tbl1a = emit_table_half(1, 0, hs1)
            tbl1b = emit_table_half(1, 1, hs1)

            partA1 = run_passA(1, tbl1a[:] if tbl1a is not None else None)
            load_idx("i2a", "ga", WGA)  # reuse pass-A idx buffer for layer 2
            accA1 = emit_canon(1, partA1)

            # pass B of layer 1, with layer-2 dense + AG2A pipelined in
            partB1 = pbpool.tile([P, MT, HID], dt.bfloat16, tag="pb")
            hs2 = hspool.tile([P, MT, HID], dt.bfloat16, tag="hs")
            AG2A_TRIG = min(MT, MTA + max(3, MT // 5))
            l2state = {}

            def l1_chunk(c0, c1):
                for t in range(c0, c1):
                    emit_dense2_tile(t, partB1, hs2)

            def l1_progress(upto):
                if upto >= AG2A_TRIG and "t2a" not in l2state:
                    l2state["t2a"] = emit_table_half(2, 0, hs2)

            run_passB(1, partB1, accA1, tbl1b[:] if tbl1b is not None else None,
                      l1_chunk, l1_progress)
            load_idx("i2b", "gb", WGB)  # reuse pass-B idx buffer for layer 2
            if "t2a" not in l2state:
                l2state["t2a"] = emit_table_half(2, 0, hs2)
            tbl2a = l2state["t2a"]
            tbl2b = emit_table_half(2, 1, hs2)

            # ================= layer 2 =================
            partA2 = run_passA(2, tbl2a[:] if tbl2a is not None else None)
            accA2 = emit_canon(2, partA2)

            partB2 = pbpool.tile([P, MT, HID], dt.bfloat16, tag="pb")

            def l2_chunk(c0, c1):
                for t in range(c0, c1):
                    pst = pstpool.tile([P, P], dt.bfloat16, tag="tr")
                    nc.tensor.transpose(pst[:], partB2[:, t, :], identb[:])
                    hT = tpool.tile([P, P], dt.bfloat16, tag="hT")
                    nc.scalar.copy(hT[:], pst[:])
                    ps2 = pspool.tile([P, CLS], dt.float32, tag="mm2")
                    nc.tensor.matmul(ps2[:], hT[:], wfcsb[:],
                                     start=True, stop=True)
                    nc.vector.tensor_add(outsb[:, t, :], ps2[:], bfcsb[:])
                flush_out(c1)

            run_passB(2, partB2, accA2, tbl2b[:] if tbl2b is not None else None,
                      l2_chunk, lambda upto: None)
            flush_out(MT)

    nc.compile()
    return nc


# ----------------------------------------------------------------------------
# Entry point
# ----------------------------------------------------------------------------

_CACHE = {}


def _graph_key(meta):
    return (meta["IN"], meta["HID"], meta["CLS"], meta["BLK"],
            meta["KA1"], meta["KB1"], meta["KA2"], meta["KB2"])


def kernel(x, edge_index, W1, b1, W2, b2, Wfc, bfc, _want_profile=False):
    x = np.asarray(x, dtype=np.float32)
    in_maps, meta = _preprocess(np.asarray(x), np.asarray(edge_index),
                                np.asarray(W1), np.asarray(b1),
                                np.asarray(W2), np.asarray(b2),
                                np.asarray(Wfc), np.asarray(bfc))
    key = _graph_key(meta)
    if key not in _CACHE:
        _CACHE[key] = _build(meta)
    nc = _CACHE[key]
    res = run_bass_kernel_spmd(nc, in_maps, core_ids=list(range(N_CORES)),
                               trace=_want_profile)
    N, CLS = meta["N"], meta["CLS"]
    BLK_RAW = meta["BLK_RAW"]
    out_pos = meta["out_pos"]
    full = np.empty((N, CLS), dtype=np.float32)
    for r in range(N_CORES):
        lo = r * BLK_RAW
        hi = min(N, (r + 1) * BLK_RAW)
        if hi > lo:
            rows = out_pos[r, 1:1 + hi - lo]
            full[lo:hi] = res.results[r]["out"][rows]
    if _want_profile:
        return full, res
    return full


# revision 8
# speedup vs baseline: 1.7444x; 1.7444x over previous
"""Trainium2 Bass kernel for a 2-layer GCN + linear classifier (PyG GCNConv style).

Self-contained: hardcodes the 8-core sharding strategy; all graph/index
preprocessing is host-side numpy, all FLOPs on x run on device.

Sharding: nodes are split into 8 contiguous blocks (one per core, padded to
a multiple of 128; slot 0 / tail slots are guaranteed-zero pad rows). Per
GCN layer each core computes its block's dense transform (bf16 PE matmuls,
f32 PSUM), pre-scales rows by dinv, and two AllGathers materialize the full
bf16 node table in every core's HBM, split into halves A/B so gather
indices fit int16. Self-loop edges are folded into the gather grids. Each
core aggregates its destinations' in-edges with gpsimd dma_gather (256B
bf16 rows, <=1024 idxs/instruction — hard ucode limit — over 4 SWDGE
queues) + DVE tree reductions, in two passes by table half. Destination
lanes are degree-sorted per (core, half); pass-A partials are re-permuted
into pass-B lane order via one small gather that overlaps pass B, so the
merge (+bias, x dinv, ReLU) runs per-tile as pass B completes and the next
layer's dense transform / AllGather pipeline into the gather tail. Layer 2
operates in pass-B-of-layer-1 row order; the final row permutation is
undone on the host. The bottleneck is Pool-engine descriptor generation at
~2.1 ns/index.
"""

import sys
import types

import numpy as np


def _setup_env():
    if "/opt/trn_rl_repo" not in sys.path:
        sys.path.insert(0, "/opt/trn_rl_repo")
    if "antenv.axon_hooks" not in sys.modules:
        try:
            from trn_agent_boot.trn_boot import _ntff_profile_via_ctypes

            _hook = _ntff_profile_via_ctypes("/opt/axon/libaxon_pjrt.so")
        except Exception:
            _hook = None
        _mod = types.ModuleType("antenv.axon_hooks")
        _mod.get_axon_ntff_profile_hook = lambda: _hook
        _mod.set_axon_ntff_profile_hook = lambda h: None
        sys.modules["antenv.axon_hooks"] = _mod


_setup_env()

import ml_dtypes  # noqa: E402
from concourse import bacc, bass, mybir, tile  # noqa: E402
import concourse.bass_utils as bass_utils  # noqa: E402
from concourse.bass_utils import run_bass_kernel_spmd  # noqa: E402
from concourse.masks import make_identity  # noqa: E402

bass_utils.upload_artifacts = lambda tmpdir: tmpdir

# --- queue-aware DMASW semaphore lane assignment -----------------------------
# Tile assigns Pool-engine DMA instructions to the 8 DMASW semaphore lanes
# round-robin in *scheduled* order, but each lane gets locked to the SWDGE
# queue of the first instruction using it. With multi-queue dma_gather this
# races; pin each queue to its own lane subset instead.
import concourse.tile_sem_assignment as _tsa  # noqa: E402
from concourse.bass_isa import UserSyncedRemoteDMADescs as _URD  # noqa: E402
from concourse.tile_sem_assignment import DMAInst as _DMAInst  # noqa: E402

_orig_assign_tick = _tsa.TileClockTick._assign_tick


def _queue_aware_assign_tick(self, inst):
    if (
        isinstance(inst, _DMAInst)
        and not isinstance(inst, _URD)
        and inst.engine == mybir.EngineType.Pool
    ):
        q = getattr(inst, "queue_num", 0) or 0
        lanes = max(1, self.swdge_sem_count // NQ)
        rot = self.__dict__.setdefault("_q_lane_rot", {})
        r = rot.get(q, 0)
        self.next_sw_dma_idx = (q * lanes + r) % self.swdge_sem_count
        rot[q] = (r + 1) % lanes
    return _orig_assign_tick(self, inst)


_tsa.TileClockTick._assign_tick = _queue_aware_assign_tick
# -----------------------------------------------------------------------------

import os  # noqa: E402

N_CORES = 8
P = 128
CHUNK = int(os.environ.get("KCHUNK", "8"))   # slabs per dma_gather instruction
NQ = int(os.environ.get("KNQ", "4"))         # SWDGE queues
GBUFS = int(os.environ.get("KGBUFS", "8"))
SCAP = int(os.environ.get("KSCAP", "40"))    # staging slabs per group
QCH = int(os.environ.get("KQCH", "12"))      # out-DMA tile chunk
KPREP = int(os.environ.get("KPREP", "0"))    # prepare_only prefix instrs (L1 passA)

dt = mybir.dt
BF16 = ml_dtypes.bfloat16


# ----------------------------------------------------------------------------
# Host-side preprocessing
# ----------------------------------------------------------------------------

def _wrap16(flat: np.ndarray) -> np.ndarray:
    """Lay out an index list in dma_gather's [128, n/16] wrapped format."""
    n = flat.shape[0]
    assert n % 16 == 0
    w = flat.reshape(n // 16, 16).T.astype(np.int16)  # [16, n//16]
    return np.tile(w, (8, 1))  # replicate across the 8 groups of 16 partitions


def _build_layer_geom(base_pos, s_core, s_slot, d_core, d_slot, dinv_base,
                      dims):
    """Geometry for one GCN layer.

    base_pos[r, j]: canonical slot j of core r -> row position in this
    layer's base space (= the order in which h/table rows are laid out).
    dinv_base[r, p]: dinv of core r's node at base position p.

    Returns per-core grids/maps and the next layer's base_pos (= pass-B
    lane order, in which this layer's outputs are produced).
    """
    BLK, MT, HA, HB, BLK_RAW = (dims["BLK"], dims["MT"], dims["HA"],
                                dims["HB"], dims["BLK_RAW"])
    MTA = dims["MTA"]

    sp = base_pos[s_core, s_slot]                 # source base positions
    s_half = (sp >= HA).astype(np.int8)
    s_row = np.where(s_half == 1,
                     s_core * HB + (sp - HA),
                     s_core * HA + sp)            # row within half table
    dp = base_pos[d_core, d_slot]                 # dst base positions

    # per-core per-half degree sorts
    percore = []
    KA = np.zeros(MT, dtype=np.int64)
    KB = np.zeros(MT, dtype=np.int64)
    for r in range(N_CORES):
        m = d_core == r
        halves = []
        p0 = base_pos[r, 0]                       # guaranteed-zero pad slot
        pt = base_pos[r, BLK_RAW + 1] if BLK_RAW + 1 < BLK else None
        for h in (0, 1):
            mh = m & (s_half == h)
            cnt = np.bincount(dp[mh], minlength=BLK)
            key = cnt.astype(np.float64)
            if h == 1:
                # force zero rows of the *next* layer's table to fixed spots:
                # pad slot 0 -> pos 0 (half A), one tail pad -> pos BLK-1 (B)
                key[p0] = -1.0
                if pt is not None:
                    key[pt] = np.inf
            perm = np.argsort(key, kind="stable")  # perm[pos] = base position
            inv = np.empty(BLK, dtype=np.int64)
            inv[perm] = np.arange(BLK)
            scnt = cnt[perm]
            Kt = scnt.reshape(MT, P).max(axis=1)
            if h == 0:
                KA = np.maximum(KA, Kt)
            else:
                KB = np.maximum(KB, Kt)
            halves.append(dict(perm=perm, inv=inv))
        percore.append(halves)

    WA, WB = int(KA.sum()), int(KB.sum())
    offA = np.concatenate([[0], np.cumsum(KA)[:-1]])
    offB = np.concatenate([[0], np.cumsum(KB)[:-1]])

    # grids (linear slab streams), fillers point at guaranteed-zero rows
    grids = []
    for r in range(N_CORES):
        m = d_core == r
        cg = []
        for h, (K, off, W) in enumerate(((KA, offA, WA), (KB, offB, WB))):
            if W == 0:
                cg.append(None)
                continue
            zrow = r * HA if h == 0 else r * HB + HB - 1
            grid = np.full((W, P), zrow, dtype=np.int64)
            mh = m & (s_half == h)
            pos = percore[r][h]["inv"][dp[mh]]
            rows = s_row[mh]
            order = np.argsort(pos, kind="stable")
            pos_s = pos[order]
            rows_s = rows[order]
            counts = np.bincount(pos_s, minlength=BLK)
            starts = np.concatenate([[0], np.cumsum(counts)[:-1]])
            k = np.arange(len(pos_s)) - starts[pos_s]
            tile_i = pos_s // P
            lane = pos_s % P
            grid[off[tile_i] + k, lane] = rows_s
            cg.append(grid)
        grids.append(cg)

    # group segmentation for the staged tree-reduce (shared geometry)
    def segments(K, off, W):
        segs = []  # per group: list of (tile, s0_in_group, length, first)
        for g0 in range(0, W, SCAP):
            g1 = min(W, g0 + SCAP)
            out = []
            t = int(np.searchsorted(off, g0, side="right")) - 1
            while t < MT and off[t] + K[t] <= g0:
                t += 1
            s = g0
            while s < g1 and t < MT:
                e = min(g1, int(off[t] + K[t]))
                if e > s:
                    out.append((t, s - g0, e - s, s == off[t]))
                s = e
                t += 1
            segs.append(out)
        return segs

    segA = segments(KA, offA, WA)
    segB = segments(KB, offB, WB)

    # pass-A -> pass-B lane-space map (per core), wrapped for dma_gather
    maps = []
    for r in range(N_CORES):
        permB = percore[r][1]["perm"]
        invA = percore[r][0]["inv"]
        maps.append(invA[permB])  # mapAB[pB] = pass-A position of that node

    # next layer's base space = pass-B order of this layer
    base_pos2 = np.empty_like(base_pos)
    dinv_base2 = np.empty_like(dinv_base)
    for r in range(N_CORES):
        invB = percore[r][1]["inv"]
        permB = percore[r][1]["perm"]
        base_pos2[r] = invB[base_pos[r]]
        dinv_base2[r] = dinv_base[r][permB]

    return dict(KA=tuple(int(x) for x in KA), KB=tuple(int(x) for x in KB),
                WA=WA, WB=WB, segA=segA, segB=segB, grids=grids, maps=maps,
                base_pos2=base_pos2, dinv_base2=dinv_base2)


def _preprocess(x, edge_index, W1, b1, W2, b2, Wfc, bfc):
    N, IN = x.shape
    HID = W1.shape[1]
    CLS = Wfc.shape[1]
    assert IN % P == 0 and HID == P

    BLK_RAW = -(-N // N_CORES)
    BLK = -(-BLK_RAW // P) * P
    assert BLK_RAW + 2 <= BLK, "need >=2 pad slots per block"
    MT = BLK // P
    # smallest half-A that keeps half-B table rows within int16: the first
    # AllGather covers less data, so pass-A gathers start earlier.
    max_half = 32767 // (N_CORES * P)
    MTA = MT - max_half if MT - max_half > 0 else MT // 2
    kmta = int(os.environ.get("KMTA", "0"))
    if kmta:
        MTA = kmta
    HA = MTA * P
    HB = BLK - HA
    NROWSA = N_CORES * HA
    NROWSB = N_CORES * HB
    assert NROWSA < 32768 and NROWSB < 32768
    assert BLK_RAW + 1 >= HA, "tail pads must land in half B"
    dims = dict(BLK=BLK, MT=MT, MTA=MTA, HA=HA, HB=HB, BLK_RAW=BLK_RAW)

    # edge list WITH self loops
    loop = np.arange(N, dtype=np.int64)
    src = np.concatenate([edge_index[0].astype(np.int64), loop])
    dst = np.concatenate([edge_index[1].astype(np.int64), loop])

    deg = np.bincount(dst, minlength=N).astype(np.float64)
    dinv = np.where(deg > 0, 1.0 / np.sqrt(np.maximum(deg, 1.0)), 0.0)
    dinv = dinv.astype(np.float32)

    s_core = src // BLK_RAW
    s_slot = 1 + src % BLK_RAW
    d_core = dst // BLK_RAW
    d_slot = 1 + dst % BLK_RAW

    # canonical (layer-1) base space: position = slot
    base_pos1 = np.tile(np.arange(BLK, dtype=np.int64)[None, :], (N_CORES, 1))
    dinv_base1 = np.zeros((N_CORES, BLK), dtype=np.float32)
    for r in range(N_CORES):
        lo = r * BLK_RAW
        hi = min(N, (r + 1) * BLK_RAW)
        if hi > lo:
            dinv_base1[r, 1:1 + hi - lo] = dinv[lo:hi]

    g1 = _build_layer_geom(base_pos1, s_core, s_slot, d_core, d_slot,
                           dinv_base1, dims)
    g2 = _build_layer_geom(g1["base_pos2"], s_core, s_slot, d_core, d_slot,
                           g1["dinv_base2"], dims)

    # output row of canonical slot j = pass-B-of-layer-2 position
    out_pos = g2["base_pos2"]  # [r, slot] -> final row

    # per-core input tensors
    b1r = np.tile(np.asarray(b1, np.float32)[None, :], (P, 1)).astype(BF16)
    b2r = np.tile(np.asarray(b2, np.float32)[None, :], (P, 1)).astype(BF16)
    bfcr = np.tile(np.asarray(bfc, np.float32)[None, :], (P, 1))
    w1 = np.asarray(W1, np.float32).astype(BF16)
    w2 = np.asarray(W2, np.float32).astype(BF16)
    wfc = np.asarray(Wfc, np.float32).astype(BF16)

    in_maps = []
    for r in range(N_CORES):
        lo = r * BLK_RAW
        hi = min(N, (r + 1) * BLK_RAW)
        xb = np.zeros((BLK, IN), dtype=np.float32)
        if hi > lo:
            xb[1:1 + hi - lo] = x[lo:hi]
        xt = np.ascontiguousarray(xb.T).astype(BF16)

        def dvt(v):  # [BLK] -> [P, MT]
            return np.ascontiguousarray(v.reshape(MT, P).T.copy())

        im = {
            "xt": xt, "w1": w1, "w2": w2, "wfc": wfc,
            "b1r": b1r, "b2r": b2r, "bfcr": bfcr,
            "dv1": dvt(dinv_base1[r]),
            "dvb1": dvt(g1["dinv_base2"][r]),
            "dvb2": dvt(g2["dinv_base2"][r]),
        }
        for li, g in ((1, g1), (2, g2)):
            for h, nm in ((0, "a"), (1, "b")):
                grid = g["grids"][r][h]
                if grid is not None:
                    im[f"i{li}{nm}"] = np.ascontiguousarray(
                        _wrap16(grid.reshape(-1)))
            im[f"m{li}"] = np.ascontiguousarray(_wrap16(g["maps"][r]))
        in_maps.append(im)

    meta = dict(N=N, IN=IN, HID=HID, CLS=CLS, BLK=BLK, BLK_RAW=BLK_RAW,
                MT=MT, MTA=MTA, NROWSA=NROWSA, NROWSB=NROWSB,
                KA1=g1["KA"], KB1=g1["KB"], KA2=g2["KA"], KB2=g2["KB"],
                segA1=g1["segA"], segB1=g1["segB"],
                segA2=g2["segA"], segB2=g2["segB"],
                WA1=g1["WA"], WB1=g1["WB"], WA2=g2["WA"], WB2=g2["WB"],
                out_pos=out_pos)
    return in_maps, meta


# ----------------------------------------------------------------------------
# Device graph
# ----------------------------------------------------------------------------

def _tree_seg(nc, g, s0, n, out_ap, acc, tpool):
    """Sum g[:, s0:s0+n, :] into out_ap (add into it if acc)."""
    HID = out_ap.shape[-1]
    if n == 1:
        if acc:
            nc.vector.tensor_add(out_ap, out_ap, g[:, s0, :])
        else:
            nc.vector.tensor_copy(out_ap, g[:, s0, :])
        return
    while n > 2:
        if n % 2 == 1:
            nc.vector.tensor_add(g[:, s0, :], g[:, s0, :], g[:, s0 + n - 1, :])
            n -= 1
            if n == 2:
                break
        h = n // 2
        nc.vector.tensor_add(g[:, s0:s0 + h, :], g[:, s0:s0 + h, :],
                             g[:, s0 + h:s0 + 2 * h, :])
        n = h
    if acc:
        nc.vector.tensor_add(g[:, s0, :], g[:, s0, :], g[:, s0 + 1, :])
        nc.vector.tensor_add(out_ap, out_ap, g[:, s0, :])
    else:
        nc.vector.tensor_add(out_ap, g[:, s0, :], g[:, s0 + 1, :])


def _build(meta):
    IN, HID, CLS = meta["IN"], meta["HID"], meta["CLS"]
    BLK, MT, MTA = meta["BLK"], meta["MT"], meta["MTA"]
    NROWSA, NROWSB = meta["NROWSA"], meta["NROWSB"]
    KC = IN // P

    nc = bacc.Bacc("TRN2", target_bir_lowering=False, debug=False,
                   num_devices=N_CORES, num_swdge_queues=NQ)

    xt = nc.dram_tensor("xt", [IN, BLK], dt.bfloat16, kind="ExternalInput")
    w1 = nc.dram_tensor("w1", [IN, HID], dt.bfloat16, kind="ExternalInput")
    w2 = nc.dram_tensor("w2", [HID, HID], dt.bfloat16, kind="ExternalInput")
    wfc = nc.dram_tensor("wfc", [HID, CLS], dt.bfloat16, kind="ExternalInput")
    b1r = nc.dram_tensor("b1r", [P, HID], dt.bfloat16, kind="ExternalInput")
    b2r = nc.dram_tensor("b2r", [P, HID], dt.bfloat16, kind="ExternalInput")
    bfcr = nc.dram_tensor("bfcr", [P, CLS], dt.float32, kind="ExternalInput")
    dv1 = nc.dram_tensor("dv1", [P, MT], dt.float32, kind="ExternalInput")
    dvb1 = nc.dram_tensor("dvb1", [P, MT], dt.float32, kind="ExternalInput")
    dvb2 = nc.dram_tensor("dvb2", [P, MT], dt.float32, kind="ExternalInput")
    idxt = {}
    for li in (1, 2):
        for nm, W in (("a", meta[f"WA{li}"]), ("b", meta[f"WB{li}"])):
            if W:
                idxt[f"i{li}{nm}"] = nc.dram_tensor(
                    f"i{li}{nm}", [P, W * 8], dt.int16, kind="ExternalInput")
        idxt[f"m{li}"] = nc.dram_tensor(
            f"m{li}", [P, BLK // 16], dt.int16, kind="ExternalInput")
    out = nc.dram_tensor("out", [BLK, CLS], dt.float32, kind="ExternalOutput")

    with tile.TileContext(nc) as tc:
        with (
            tc.tile_pool(name="const", bufs=1) as cpool,
            tc.tile_pool(name="idx", bufs=1) as ipool,
            tc.tile_pool(name="hs", bufs=1) as hspool,
            tc.tile_pool(name="pa", bufs=1) as papool,
            tc.tile_pool(name="acc", bufs=1) as accpool,
            tc.tile_pool(name="pb", bufs=2) as pbpool,
            tc.tile_pool(name="outs", bufs=1) as opool,
            tc.tile_pool(name="xload", bufs=3) as xpool,
            tc.tile_pool(name="gbuf", bufs=GBUFS) as gpool,
            tc.tile_pool(name="lhsT", bufs=3) as tpool,
            tc.tile_pool(name="ps", bufs=3, space="PSUM") as pspool,
            tc.tile_pool(name="pst", bufs=2, space="PSUM") as pstpool,
            tc.tile_pool(name="dram", bufs=1, space="DRAM") as dpool,
        ):
            # ---- constants ----
            w1sb = cpool.tile([P, KC, HID], dt.bfloat16, tag="w1")
            nc.sync.dma_start(out=w1sb[:], in_=w1[:].rearrange("(c k) h -> k c h", k=P))
            w2sb = cpool.tile([P, HID], dt.bfloat16, tag="w2")
            nc.sync.dma_start(out=w2sb[:], in_=w2[:])
            wfcsb = cpool.tile([P, CLS], dt.bfloat16, tag="wfc")
            nc.sync.dma_start(out=wfcsb[:], in_=wfc[:])
            b1sb = cpool.tile([P, HID], dt.bfloat16, tag="b1")
            nc.sync.dma_start(out=b1sb[:], in_=b1r[:])
            b2sb = cpool.tile([P, HID], dt.bfloat16, tag="b2")
            nc.sync.dma_start(out=b2sb[:], in_=b2r[:])
            bfcsb = cpool.tile([P, CLS], dt.float32, tag="bfc")
            nc.sync.dma_start(out=bfcsb[:], in_=bfcr[:])
            dvsb = {}
            for nm, t in (("dv1", dv1), ("dvb1", dvb1), ("dvb2", dvb2)):
                s = cpool.tile([P, MT], dt.float32, tag=nm)
                nc.sync.dma_start(out=s[:], in_=t[:])
                dvsb[nm] = s
            ident = cpool.tile([P, P], dt.float32, tag="ident")
            make_identity(nc, ident[:])
            identb = cpool.tile([P, P], dt.bfloat16, tag="identb")
            nc.vector.tensor_copy(identb[:], ident[:])

            # Index buffers are shared between layers (layer-2 tables stream
            # in mid-run once layer-1 passes finish with the buffer).
            WGA = 8 * max(meta["WA1"], meta["WA2"])
            WGB = 8 * max(meta["WB1"], meta["WB2"])
            isb = {}

            def load_idx(nm, tag, width):
                t = idxt.get(nm)
                if t is None:
                    return
                s = ipool.tile([P, width], dt.int16, tag=tag)
                # Activation-engine HWDGE queue: keeps ~7MB of index tables
                # from delaying the layer-1 x tile loads on the Sync queue.
                nc.scalar.dma_start(out=s[:, :t.shape[1]], in_=t[:])
                isb[nm] = s

            load_idx("i1a", "ga", WGA)
            load_idx("i1b", "gb", WGB)
            load_idx("m1", "m1", BLK // 16)
            load_idx("m2", "m2", BLK // 16)

            self_q = [0]  # rotating SWDGE queue assignment
            outsb = opool.tile([P, MT, CLS], dt.float32, tag="outsb")
            out_flushed = [0]

            def flush_out(upto):
                if upto > out_flushed[0]:
                    m0 = out_flushed[0]
                    nc.sync.dma_start(
                        out=out[:].rearrange("(t p) c -> p t c", p=P)[:, m0:upto, :],
                        in_=outsb[:, m0:upto, :],
                    )
                    out_flushed[0] = upto

            prep_sems = ([nc.alloc_semaphore(f"gprep{q}") for q in range(NQ)]
                         if KPREP > 0 else None)

            def run_pass(Wtot, segs, isbuf, tview, part, Ks, prep_n=0):
                # memset zero-K tiles once
                for t in range(MT):
                    if Ks[t] == 0:
                        nc.vector.memset(part[:, t, :], 0.0)
                emitted = [0]
                pending = set()
                for gi, seglist in enumerate(segs):
                    g0 = gi * SCAP
                    glen = min(SCAP, Wtot - g0)
                    gt = gpool.tile([P, SCAP, HID], dt.bfloat16, tag="g")
                    g = gt[:]
                    s0 = 0
                    while s0 < glen:
                        kc = min(CHUNK, glen - s0)
                        o0 = g0 + s0
                        q = self_q[0] % NQ
                        if emitted[0] < prep_n:
                            nc.gpsimd.dma_gather(
                                out_ap=g[:, s0:s0 + kc, :],
                                in_ap=tview,
                                idxs_ap=isbuf[:, o0 * 8:(o0 + kc) * 8],
                                num_idxs=kc * P,
                                num_idxs_reg=kc * P,
                                elem_size=HID,
                                queue_num=q,
                                prepare_only=True,
                                sem=prep_sems[q],
                            )
                            pending.add(q)
                        else:
                            if pending:
                                for qq in sorted(pending):
                                    nc.gpsimd.trigger_dma(count=None,
                                                          queue_num=qq)
                                pending.clear()
                            nc.gpsimd.dma_gather(
                                out_ap=g[:, s0:s0 + kc, :],
                                in_ap=tview,
                                idxs_ap=isbuf[:, o0 * 8:(o0 + kc) * 8],
                                num_idxs=kc * P,
                                num_idxs_reg=kc * P,
                                elem_size=HID,
                                queue_num=q,
                            )
                        emitted[0] += 1
                        self_q[0] += 1
                        s0 += kc
                    for (t, ss, ln, first) in seglist:
                        _tree_seg(nc, g, ss, ln, part[:, t, :],
                                  not first, tpool)
                    yield gi
                if pending:
                    for qq in sorted(pending):
                        nc.gpsimd.trigger_dma(count=None, queue_num=qq)
                    pending.clear()

            def emit_dense2_tile(m, h_prev, hs2):
                pst = pstpool.tile([P, P], dt.bfloat16, tag="tr")
                nc.tensor.transpose(pst[:], h_prev[:, m, :], identb[:])
                hT = tpool.tile([P, P], dt.bfloat16, tag="hT")
                nc.scalar.copy(hT[:], pst[:])
                ps = pspool.tile([P, HID], dt.float32, tag="mm")
                nc.tensor.matmul(ps[:], hT[:], w2sb[:], start=True, stop=True)
                nc.vector.tensor_scalar_mul(hs2[:, m, :], ps[:],
                                            dvsb["dvb1"][:, m:m + 1])

            def emit_table_half(layer, hseg, hs):
                t0, t1 = (0, MTA) if hseg == 0 else (MTA, MT)
                nrows = NROWSA if hseg == 0 else NROWSB
                if t1 == t0:
                    return None
                agin = dpool.tile([(t1 - t0) * P, HID], dt.bfloat16,
                                  tag=f"agin{layer}{hseg}")
                # Activation-engine HWDGE queue: avoids queueing behind the
                # x-tile loads on Sync.
                nc.scalar.dma_start(
                    out=agin[:].rearrange("(t p) h -> p t h", p=P),
                    in_=hs[:, t0:t1, :],
                )
                tbl = dpool.tile([nrows, HID], dt.bfloat16,
                                 tag=f"table{layer}{hseg}",
                                 addr_space="Shared")
                nc.gpsimd.collective_compute(
                    "AllGather",
                    mybir.AluOpType.bypass,
                    replica_groups=[list(range(N_CORES))],
                    ins=[agin[:].opt()],
                    outs=[tbl[:].opt()],
                )
                return tbl

            def emit_canon(layer, partA):
                pdA = dpool.tile([BLK, HID], dt.bfloat16, tag=f"pd{layer}")
                nc.sync.dma_start(
                    out=pdA[:].rearrange("(t p) h -> p t h", p=P), in_=partA[:]
                )
                accA = accpool.tile([P, MT, HID], dt.bfloat16, tag="acc")
                msb = isb[f"m{layer}"]
                for c0 in range(0, MT, CHUNK):
                    cc = min(CHUNK, MT - c0)
                    nc.gpsimd.dma_gather(
                        out_ap=accA[:, c0:c0 + cc, :], in_ap=pdA[:],
                        idxs_ap=msb[:, c0 * 8:(c0 + cc) * 8],
                        num_idxs=cc * P, num_idxs_reg=cc * P,
                        elem_size=HID, queue_num=self_q[0] % NQ,
                    )
                    self_q[0] += 1
                return accA

            MCH = int(os.environ.get("KMCH", "7"))  # merge chunk width (tiles)

            def run_passB(layer, partB, accA, tview, on_chunk, on_progress):
                dmrg = dvsb["dvb1" if layer == 1 else "dvb2"]
                bsb = b1sb if layer == 1 else b2sb
                KB = meta[f"KB{layer}"]
                segB = meta[f"segB{layer}"]
                WB = meta[f"WB{layer}"]
                merged_upto = [0]

                def merge_chunk(c0, c1):
                    w = c1 - c0
                    sl = slice(c0, c1)
                    nc.vector.tensor_add(partB[:, sl, :], partB[:, sl, :],
                                         accA[:, sl, :])
                    dv3 = dmrg[:, sl].to_broadcast([P, w, HID])
                    nc.vector.tensor_tensor(partB[:, sl, :], partB[:, sl, :],
                                            dv3, op=mybir.AluOpType.mult)
                    b3 = bsb[:].rearrange("p (o h) -> p o h", o=1).to_broadcast(
                        [P, w, HID])
                    nc.vector.tensor_tensor(partB[:, sl, :], partB[:, sl, :],
                                            b3, op=mybir.AluOpType.add)
                    nc.scalar.activation(partB[:, sl, :], partB[:, sl, :],
                                         mybir.ActivationFunctionType.Relu)
                    on_chunk(c0, c1)

                def merge_ready(done_tiles):
                    while merged_upto[0] < done_tiles:
                        c0 = merged_upto[0]
                        c1 = min(c0 + MCH, MT)
                        if c1 > done_tiles:
                            break
                        merge_chunk(c0, c1)
                        merged_upto[0] = c1
                        on_progress(merged_upto[0])

                offB = np.concatenate([[0], np.cumsum(KB)[:-1]])
                if WB:
                    for gi in run_pass(WB, segB, isb[f"i{layer}b"],
                                       tview, partB, KB):
                        done_upto = (gi + 1) * SCAP
                        nd = 0
                        while nd < MT and offB[nd] + KB[nd] <= done_upto:
                            nd += 1
                        merge_ready(nd)
                else:
                    for t in range(MT):
                        nc.vector.memset(partB[:, t, :], 0.0)
                merge_ready(MT)

            def run_passA(layer, tview):
                KA = meta[f"KA{layer}"]
                segA = meta[f"segA{layer}"]
                WA = meta[f"WA{layer}"]
                partA = papool.tile([P, MT, HID], dt.bfloat16, tag="pa")
                if WA:
                    pn = KPREP if layer == 1 else 0
                    for _ in run_pass(WA, segA, isb[f"i{layer}a"],
                                      tview, partA, KA, prep_n=pn):
                        pass
                else:
                    for t in range(MT):
                        nc.vector.memset(partA[:, t, :], 0.0)
                return partA

            # ================= layer 1 =================
            hs1 = hspool.tile([P, MT, HID], dt.bfloat16, tag="hs")
            for m in range(MT):
                ps = pspool.tile([P, HID], dt.float32, tag="mm")
                xm = xpool.tile([P, KC, P], dt.bfloat16, tag="x")
                nc.sync.dma_start(
                    out=xm[:],
                    in_=xt[:].rearrange("(c k) m -> k c m", k=P)[
                        :, :, m * P:(m + 1) * P
                    ],
                )
                for c in range(KC):
                    nc.tensor.matmul(ps[:], xm[:, c, :], w1sb[:, c, :],
                                     start=(c == 0), stop=(c == KC - 1))
                nc.vector.tensor_scalar_mul(hs1[:, m, :], ps[:],
                                            dvsb["dv1"][:, m:m + 1])
            tbl1a = emit_table_half(1, 0, hs1)
            tbl1b = emit_table_half(1, 1, hs1)

            partA1 = run_passA(1, tbl1a[:] if tbl1a is not None else None)
            load_idx("i2a", "ga", WGA)  # reuse pass-A idx buffer for layer 2
            accA1 = emit_canon(1, partA1)

            # pass B of layer 1, with layer-2 dense + AG2A pipelined in
            partB1 = pbpool.tile([P, MT, HID], dt.bfloat16, tag="pb")
            hs2 = hspool.tile([P, MT, HID], dt.bfloat16, tag="hs")
            AG2A_TRIG = min(MT, MTA + max(3, MT // 5))
            l2state = {}

            def l1_chunk(c0, c1):
                for t in range(c0, c1):
                    emit_dense2_tile(t, partB1, hs2)

            def l1_progress(upto):
                if upto >= AG2A_TRIG and "t2a" not in l2state:
                    l2state["t2a"] = emit_table_half(2, 0, hs2)

            run_passB(1, partB1, accA1, tbl1b[:] if tbl1b is not None else None,
                      l1_chunk, l1_progress)
            load_idx("i2b", "gb", WGB)  # reuse pass-B idx buffer for layer 2
            if "t2a" not in l2state:
                l2state["t2a"] = emit_table_half(2, 0, hs2)
            tbl2a = l2state["t2a"]
            tbl2b = emit_table_half(2, 1, hs2)

            # ================= layer 2 =================
            partA2 = run_passA(2, tbl2a[:] if tbl2a is not None else None)
            accA2 = emit_canon(2, partA2)

            partB2 = pbpool.tile([P, MT, HID], dt.bfloat16, tag="pb")

            def l2_chunk(c0, c1):
                for t in range(c0, c1):
                    pst = pstpool.tile([P, P], dt.bfloat16, tag="tr")
                    nc.tensor.transpose(pst[:], partB2[:, t, :], identb[:])
                    hT = tpool.tile([P, P], dt.bfloat16, tag="hT")
                    nc.scalar.copy(hT[:], pst[:])
                    ps2 = pspool.tile([P, CLS], dt.float32, tag="mm2")
                    nc.tensor.matmul(ps2[:], hT[:], wfcsb[:],
                                     start=True, stop=True)
                    nc.vector.tensor_add(outsb[:, t, :], ps2[:], bfcsb[:])
                flush_out(c1)

            run_passB(2, partB2, accA2, tbl2b[:] if tbl2b is not None else None,
                      l2_chunk, lambda upto: None)
            flush_out(MT)

    nc.compile()
    return nc


# ----------------------------------------------------------------------------
# Entry point
# ----------------------------------------------------------------------------

_CACHE = {}


def _graph_key(meta):
    return (meta["IN"], meta["HID"], meta["CLS"], meta["BLK"],
            meta["KA1"], meta["KB1"], meta["KA2"], meta["KB2"])


def kernel(x, edge_index, W1, b1, W2, b2, Wfc, bfc, _want_profile=False):
    x = np.asarray(x, dtype=np.float32)
    in_maps, meta = _preprocess(np.asarray(x), np.asarray(edge_index),
                                np.asarray(W1), np.asarray(b1),
                                np.asarray(W2), np.asarray(b2),
                                np.asarray(Wfc), np.asarray(bfc))
    key = _graph_key(meta)
    if key not in _CACHE:
        _CACHE[key] = _build(meta)
    nc = _CACHE[key]
    res = run_bass_kernel_spmd(nc, in_maps, core_ids=list(range(N_CORES)),
                               trace=_want_profile)
    N, CLS = meta["N"], meta["CLS"]
    BLK_RAW = meta["BLK_RAW"]
    out_pos = meta["out_pos"]
    full = np.empty((N, CLS), dtype=np.float32)
    for r in range(N_CORES):
        lo = r * BLK_RAW
        hi = min(N, (r + 1) * BLK_RAW)
        if hi > lo:
            rows = out_pos[r, 1:1 + hi - lo]
            full[lo:hi] = res.results[r]["out"][rows]
    if _want_profile:
        return full, res
    return full
